# revision 1
# baseline (speedup 1.0000x reference)
"""Data-parallel Trainium2 kernel for nn_Attention_Fusion_31172872634407.

Sharding: batch (512) split 8 ways across the 8 NeuronCores; all params
replicated; the 26-step attentive scan is independent per example, so
there is no cross-device communication. Runs SPMD on the 8 cores via
jax.pmap on the neuron PJRT backend.
"""

import numpy as np
import jax
import jax.numpy as jnp

B, T_ENC, C, H, NSTEPS, NCLS, NEMB = 512, 64, 512, 512, 26, 97, 256
NDEV = 8
BL = B // NDEV  # 64 examples per core

_PARAM_NAMES = (
    "W_i2h", "W_h2h", "b_h2h", "w_score", "W_ih", "W_hh", "b_ih", "b_hh",
    "Wf1", "bf1", "Wf2", "bf2", "Wz", "bz", "W_gen", "b_gen", "emb_table",
)


def _local_forward(batch_H, Attentive_Sequence, text, params):
    (W_i2h, W_h2h, b_h2h, w_score, W_ih, W_hh, b_ih, b_hh,
     Wf1, bf1, Wf2, bf2, Wz, bz, W_gen, b_gen, emb_table) = params
    b = batch_H.shape[0]
    char_embs = emb_table[text]                                  # [b, NSTEPS, NEMB]
    H_proj = jnp.einsum('btc,hc->bth', batch_H, W_i2h)
    h0 = jnp.zeros((b, H), jnp.float32)
    c0 = jnp.zeros((b, H), jnp.float32)

    # Precompute everything loop-invariant (keeps the sequential part lean).
    F2 = jnp.tanh(jnp.einsum('bsh,oh->bso', Attentive_Sequence, Wf2) + bf2)
    Z2 = jnp.einsum('bsh,oh->bso', Attentive_Sequence, Wz[:, H:])
    G_pre = (jnp.einsum('bse,oe->bso', char_embs, W_ih[:, C:])
             + b_ih + b_hh)                                      # [b, NSTEPS, 4H]

    def step(carry, xs):
        h, c = carry
        g_pre_t, seq_t, f2_t, z2_t = xs
        hp = h @ W_h2h.T + b_h2h
        e = jnp.einsum('bth,h->bt', jnp.tanh(H_proj + hp[:, None, :]), w_score)
        alpha = jax.nn.softmax(e, axis=1)
        context = jnp.einsum('bt,btc->bc', alpha, batch_H)
        f1 = jnp.tanh(context @ Wf1.T + bf1)
        z = jax.nn.sigmoid(context @ Wz[:, :H].T + z2_t + bz)
        I_char = z * f1 + (1.0 - z) * f2_t
        gates = I_char @ W_ih[:, :C].T + g_pre_t + h @ W_hh.T
        i_g, f_g, g_g, o_g = jnp.split(gates, 4, axis=1)
        c_new = jax.nn.sigmoid(f_g) * c + jax.nn.sigmoid(i_g) * jnp.tanh(g_g)
        h_new = jax.nn.sigmoid(o_g) * jnp.tanh(c_new)
        return (h_new, c_new), (h_new, alpha, I_char)

    xs = (jnp.swapaxes(G_pre, 0, 1), jnp.swapaxes(Attentive_Sequence, 0, 1),
          jnp.swapaxes(F2, 0, 1), jnp.swapaxes(Z2, 0, 1))
    _, (hs, alphas, chars) = jax.lax.scan(step, (h0, c0), xs)
    output_hiddens = jnp.swapaxes(hs, 0, 1)
    seq_attention_map = jnp.transpose(alphas, (1, 2, 0))
    Char = jnp.swapaxes(chars, 0, 1)
    probs = output_hiddens @ W_gen.T + b_gen
    return probs, seq_attention_map, output_hiddens, Char


_pmapped = jax.pmap(
    _local_forward,
    in_axes=(0, 0, 0, None),
    static_broadcasted_argnums=(),
)


def kernel(batch_H, Attentive_Sequence, text, **kw):
    params = tuple(jnp.asarray(np.asarray(kw[n], dtype=np.float32))
                   for n in _PARAM_NAMES)
    bh = np.asarray(batch_H, dtype=np.float32).reshape(NDEV, BL, T_ENC, C)
    as_ = np.asarray(Attentive_Sequence, dtype=np.float32).reshape(NDEV, BL, NSTEPS, H)
    tx = np.asarray(text, dtype=np.int32).reshape(NDEV, BL, NSTEPS)

    probs, attn, hid, char = _pmapped(jnp.asarray(bh), jnp.asarray(as_),
                                      jnp.asarray(tx), params)
    probs = np.asarray(probs).reshape(B, NSTEPS, NCLS)
    attn = np.asarray(attn).reshape(B, T_ENC, NSTEPS)
    hid = np.asarray(hid).reshape(B, NSTEPS, H)
    char = np.asarray(char).reshape(B, NSTEPS, H)
    return probs, attn, hid, char
